# revision 4
# baseline (speedup 1.0000x reference)
"""Trainium2 Bass kernel: per-sample hypernetwork depthwise 3x3 conv.

Reference computation (per batch b):
    W_dw[b] = (z[b] @ W_lin.T).reshape(OUT_C, 1, 3, 3)
    y[b]    = depthwise_conv2d(x[b], W_dw[b], padding=1)

Sharding: data-parallel over batch across 8 NeuronCores (2 batches/core),
W_lin replicated. Each core computes its own W_dw on-device.

Per-core kernel design:
  - channels (256) -> 2 groups of 128 on SBUF partitions
  - image processed in 32-row bands with a 1-pixel zero halo (130-wide rows)
  - 9 conv taps split between engines:
      * N_PE_TAPS taps on TensorE: diagonal-weight matmuls (float32r,
        1 cycle/row) accumulating in PSUM
      * remaining taps on VectorE: fused scalar_tensor_tensor FMAs whose
        first op also drains PSUM
  - ScalarE (ACT) performs the f32 -> f32r rounding pass in-place
  - W_dw computed on-device by 18 small fp32 matmuls from a host-side
    re-layout of W_lin (pure permutation/transpose, no host math)
"""

import os
import sys

for _p in ("/opt/trn_rl_repo", "/root/.axon_site", "/root/.axon_site/_ro/trn_rl_repo",
           "/root/.axon_site/_ro/pypackages"):
    if os.path.isdir(_p) and _p not in sys.path:
        sys.path.append(_p)

import numpy as np

import concourse.bass as bass
import concourse.tile as tile
from concourse import bacc, mybir
from concourse import bass_utils
from concourse.alu_op_type import AluOpType

F32 = mybir.dt.float32
F32R = mybir.dt.float32r

# problem constants (hardcoded per contract)
B, OUT_C, H, W = 16, 256, 128, 128
K, Z_DIM = 3, 64
N_CORES = 8
B_PER = B // N_CORES          # 2 batches per core
G = OUT_C // 128              # 2 channel groups of 128

# tuning knobs
N_PE_TAPS = 6                 # taps 0..N_PE_TAPS-1 on TensorE (f32r), rest on DVE (f32)
ROWS_BAND = 32                # image rows per SBUF band
ROWS_GROUP = 16               # image rows per PSUM accumulation group
PE_EXACT_F32 = False          # True -> plain fp32 matmuls (4 cyc/row), no rounding

PADW = W + 2                  # 130
TAPS = [(dy, dx) for dy in range(3) for dx in range(3)]


def build_nc(n_pe_taps=N_PE_TAPS, rows_band=ROWS_BAND, rows_group=ROWS_GROUP,
             exact=PE_EXACT_F32, b_per=B_PER, h=H):
    """Build the per-core Bass program. Returns compiled Bacc object."""
    pe_taps = TAPS[:n_pe_taps]
    dve_taps = TAPS[n_pe_taps:]
    n_bands = h // rows_band
    grp_per_band = rows_band // rows_group
    banks_per_grp = rows_group * W // 512     # 512 f32 per PSUM bank
    grp_free = rows_group * W                 # free elems per psum group

    nc = bacc.Bacc("TRN2", target_bir_lowering=False, debug=False)

    x_d = nc.dram_tensor("x", [b_per, OUT_C, h, W], F32, kind="ExternalInput")
    zt_d = nc.dram_tensor("zT", [Z_DIM, b_per], F32, kind="ExternalInput")
    wlt_d = nc.dram_tensor("wlt", [Z_DIM, OUT_C * K * K], F32, kind="ExternalInput")
    ident_d = nc.dram_tensor("ident", [128, 128], F32, kind="ExternalInput")
    y_d = nc.dram_tensor("y", [b_per, OUT_C, h, W], F32, kind="ExternalOutput")

    n_chunks = OUT_C * K * K // 128           # 18 chunks of 128 along (t, g, c)
    wd_cols = K * K * G * b_per               # 36 = (t, g, b)

    pe_dt = F32 if exact else F32R

    with tile.TileContext(nc) as tc:
        # ---------- prologue: W_dw = z @ W_lin.T, laid out [c_p, (t, g, b)] ----------
        with tc.tile_pool(name="wconst", bufs=1) as wpool:
            ident = wpool.tile([128, 128], F32)
            nc.sync.dma_start(ident[:], ident_d.ap()[:, :])
            wlt = wpool.tile([Z_DIM, OUT_C * K * K], F32)
            nc.sync.dma_start(wlt[:], wlt_d.ap()[:, :])
            zt = wpool.tile([Z_DIM, b_per], F32)
            nc.sync.dma_start(zt[:], zt_d.ap()[:, :])

            wd = wpool.tile([128, wd_cols], F32)
            with tc.tile_pool(name="wpsum", bufs=2, space="PSUM") as wps:
                for j in range(n_chunks):
                    ps = wps.tile([128, b_per], F32)
                    nc.tensor.matmul(ps[:], wlt[:, 128 * j:128 * (j + 1)], zt[:],
                                     start=True, stop=True)
                    nc.vector.tensor_copy(wd[:, b_per * j:b_per * (j + 1)], ps[:])

            # diagonal weight matrices for the PE taps
            diags = {}
            for b in range(b_per):
                for g in range(G):
                    for ti, _ in enumerate(pe_taps):
                        col = (ti * G + g) * b_per + b
                        dtile = wpool.tile([128, 128], pe_dt, tag=f"diag_{b}_{g}_{ti}")
                        nc.vector.tensor_scalar(
                            out=dtile[:], in0=ident[:],
                            scalar1=wd[:, col:col + 1], scalar2=None,
                            op0=AluOpType.mult)
                        diags[(b, g, ti)] = dtile

            # ---------- main loop over (batch, group, band) ----------
            with tc.tile_pool(name="xband", bufs=3) as xpool, \
                 tc.tile_pool(name="xrband", bufs=3) as xrpool, \
                 tc.tile_pool(name="oband", bufs=3) as opool, \
                 tc.tile_pool(name="psum", bufs=2, space="PSUM") as pspool:
                for b in range(b_per):
                    for g in range(G):
                        for band in range(n_bands):
                            r0 = band * rows_band
                            # src rows [lo, hi) from DRAM; pad row 0 == image row r0-1
                            lo = max(r0 - 1, 0)
                            hi = min(r0 + rows_band + 1, h)
                            pad_rows = rows_band + 2

                            xt = xpool.tile([128, pad_rows * PADW], F32)
                            xv = xt[:].rearrange("p (r c) -> p r c", c=PADW)
                            # zero halo columns (and top/bottom halo rows at edges)
                            nc.gpsimd.memset(xv[:, :, 0:1], 0.0)
                            nc.gpsimd.memset(xv[:, :, PADW - 1:PADW], 0.0)
                            if r0 == 0:
                                nc.gpsimd.memset(xv[:, 0:1, 1:PADW - 1], 0.0)
                            if r0 + rows_band == h:
                                nc.gpsimd.memset(
                                    xv[:, pad_rows - 1:pad_rows, 1:PADW - 1], 0.0)

                            dst_r = lo - (r0 - 1)
                            nc.sync.dma_start(
                                xv[:, dst_r:dst_r + (hi - lo), 1:1 + W],
                                x_d.ap()[b, 128 * g:128 * (g + 1), lo:hi, :])

                            if exact:
                                xr = xv
                            else:
                                # rounded copy to f32r on ScalarE (ACT)
                                xrt = xrpool.tile([128, pad_rows * PADW], F32R)
                                nc.scalar.copy(xrt[:], xt[:])
                                xr = xrt[:].rearrange("p (r c) -> p r c", c=PADW)

                            for grp in range(grp_per_band):
                                gr0 = grp * rows_group   # band-local first out row
                                ps = pspool.tile([128, grp_free], F32)
                                for bank in range(banks_per_grp):
                                    rows_bank = 512 // W
                                    for ti in range(len(pe_taps)):
                                        dy, dx = pe_taps[ti]
                                        rs = gr0 + bank * rows_bank + dy
                                        rhs = xr[:, rs:rs + rows_bank, dx:dx + W]
                                        nc.tensor.matmul(
                                            ps[:, 512 * bank:512 * (bank + 1)],
                                            diags[(b, g, ti)][:], rhs,
                                            start=(ti == 0),
                                            stop=(ti == len(pe_taps) - 1))

                                ot = opool.tile([128, grp_free], F32)
                                acc = ps
                                for k, (dy, dx) in enumerate(dve_taps):
                                    ti = n_pe_taps + k
                                    col = (ti * G + g) * b_per + b
                                    rs = gr0 + dy
                                    in0 = xv[:, rs:rs + rows_group, dx:dx + W]
                                    nc.vector.scalar_tensor_tensor(
                                        out=ot[:], in0=in0,
                                        scalar=wd[:, col:col + 1], in1=acc[:],
                                        op0=AluOpType.mult, op1=AluOpType.add)
                                    acc = ot
                                if not dve_taps:
                                    nc.vector.tensor_copy(ot[:], ps[:])

                                nc.sync.dma_start(
                                    y_d.ap()[b, 128 * g:128 * (g + 1),
                                             r0 + gr0:r0 + gr0 + rows_group, :],
                                    ot[:])

    nc.compile()
    return nc


def make_in_maps(x, z, W_lin, b_per=B_PER):
    """Host-side shard + layout transforms (no math)."""
    # W_lin rows k = c*9 + t  ->  k' = t*256 + g*128 + c_p ; then transpose
    wl = np.asarray(W_lin, dtype=np.float32)
    wlperm = (wl.reshape(G, 128, K * K, Z_DIM)
                .transpose(2, 0, 1, 3)
                .reshape(OUT_C * K * K, Z_DIM))
    wlt = np.ascontiguousarray(wlperm.T)                  # [64, 2304]
    ident = np.eye(128, dtype=np.float32)
    x = np.asarray(x, dtype=np.float32)
    z = np.asarray(z, dtype=np.float32)
    in_maps = []
    for c in range(N_CORES):
        sl = slice(c * b_per, (c + 1) * b_per)
        in_maps.append({
            "x": np.ascontiguousarray(x[sl]),
            "zT": np.ascontiguousarray(z[sl].T),          # [64, b_per]
            "wlt": wlt,
            "ident": ident,
        })
    return in_maps


_NC_CACHE = {}


def kernel(x, z, W_lin):
    key = "main"
    if key not in _NC_CACHE:
        _NC_CACHE[key] = build_nc()
    nc = _NC_CACHE[key]
    in_maps = make_in_maps(x, z, W_lin)
    res = bass_utils.run_bass_kernel_spmd(nc, in_maps, core_ids=list(range(N_CORES)))
    out = np.concatenate([res.results[c]["y"] for c in range(N_CORES)], axis=0)
    return out.astype(np.float32, copy=False)
